# revision 10
# baseline (speedup 1.0000x reference)
"""AdaptiveLinearWithChannel on 8 TRN2 NeuronCores.

out[n] = x[n] @ weight[indices[n], t] + bias[indices[n], t]
  x: [192, 2048, 256] f32, weight: [256, 8, 256, 256] f32,
  bias: [256, 8, 1, 256] f32, indices: [192] int, t: scalar int
  out: [192, 2048, 256] f32

Sharding: selected-channel axis (192) split 24-per-core across 8 cores
(expert/data parallel — no collectives). The host gathers each core's 24
weight slices from the table (equivalent traffic to a device-side gather:
only the indexed slices ever move) and pre-transposes x so the contraction
axis lands on SBUF partitions.

Device kernel (per core, per channel n):
  out_t[oh*128+o, p] = sum_ih sum_i w[ih*128+i, oh*128+o] * xT[ih*128+i, p]
  - stationary operand = weight tile [i=128, o=128] (4 per channel)
  - moving operand = xT tile [i=128, p=512]
  - accumulate over ih in PSUM [128, 512] f32 (one bank), then drain to
    SBUF fused with the bias add (bias varies along PSUM partitions -> ACT
    per-partition bias / DVE tensor_scalar). Drains alternate ACT/DVE.
  - output written transposed; host untransposes.

Two precision modes (MODE):
  "fp16": x/w/out cross HBM as fp16 (half traffic, DMA-bound kernel);
          f32 PSUM accumulate, bias added in f32. ~1e-3 rel err.
  "f32r": all HBM traffic f32; PE runs the full-rate fp32 path (float32r,
          TF32-like). ~1.5e-4 rel err, ~2x slower (DMA-bound at f32 bytes).
"""

import numpy as np

MODE = "fp16"  # "fp16" | "f32r"

N_CORES = 8
N_SEL = 192
N_CH = N_SEL // N_CORES  # 24 channels per core
NPT = 2048               # points per channel
CIN = 256
COUT = 256
P = 128                  # SBUF/PSUM partitions
PC = 512                 # moving-operand chunk (one PSUM bank of f32)
N_PC = NPT // PC         # 4
X_BUFS = 6
O_BUFS = 6
W_BUFS = 6

_CACHE = {}


def _build(mode):
    import concourse.mybir as mybir
    import concourse.tile as tile
    from concourse import bacc

    f32 = mybir.dt.float32
    if mode == "fp16":
        io_dt = mybir.dt.float16   # dtype of x/w/out in DRAM and SBUF
        pe_dt = mybir.dt.float16   # dtype the PE sees
    else:
        io_dt = f32
        pe_dt = mybir.dt.float32r

    def pe_cast(ap):
        return ap.bitcast(pe_dt) if ap.dtype != pe_dt else ap

    nc = bacc.Bacc(None, target_bir_lowering=False)
    xt_d = nc.dram_tensor("xt", [N_CH, 2, P, NPT], io_dt, kind="ExternalInput")
    wt_d = nc.dram_tensor("wt", [N_CH, 2, P, COUT], io_dt, kind="ExternalInput")
    bt_d = nc.dram_tensor("bt", [2, P, N_CH], f32, kind="ExternalInput")
    out_d = nc.dram_tensor("out", [N_CH, 2, P, NPT], io_dt, kind="ExternalOutput")

    with tile.TileContext(nc) as tc:
        with (
            tc.tile_pool(name="xp", bufs=X_BUFS) as xp,
            tc.tile_pool(name="wp", bufs=W_BUFS) as wp,
            tc.tile_pool(name="bp", bufs=1) as bp,
            tc.tile_pool(name="op", bufs=O_BUFS) as op,
            tc.tile_pool(name="ps", bufs=4, space="PSUM") as ps,
        ):
            b_sb = bp.tile([P, 2, N_CH], f32, tag="b")
            nc.sync.dma_start(b_sb[:], bt_d.rearrange("oh o n -> o oh n"))

            for n in range(N_CH):
                x_sb = xp.tile([P, 2, NPT], pe_dt, tag="x")
                nc.sync.dma_start(
                    x_sb[:], pe_cast(xt_d[n].rearrange("ih i p -> i ih p"))
                )
                w_sb = wp.tile([P, 2, COUT], pe_dt, tag="w")
                nc.scalar.dma_start(
                    w_sb[:], pe_cast(wt_d[n].rearrange("ih i o -> i ih o"))
                )
                o_sb = op.tile([P, 2, NPT], io_dt, tag="o")
                # 4 two-bank PSUM tiles per channel; each holds 1024 output
                # columns of one o-half. Drains alternate ACT/DVE so both
                # engines share every wave.
                for tile_idx, (oh, pch) in enumerate(
                    [(0, 0), (0, 1), (1, 0), (1, 1)]
                ):
                    bias_ap = b_sb[:, oh, n : n + 1]
                    acc = ps.tile([P, 2 * PC], f32, tag="acc")
                    for pc2 in range(2):
                        pcg = pch * 2 + pc2
                        for ih in range(2):
                            nc.tensor.matmul(
                                acc[:, pc2 * PC : (pc2 + 1) * PC],
                                w_sb[:, ih, oh * P : (oh + 1) * P],
                                x_sb[:, ih, pcg * PC : (pcg + 1) * PC],
                                start=(ih == 0),
                                stop=(ih == 1),
                            )
                    dst = o_sb[:, oh, pch * 2 * PC : (pch + 1) * 2 * PC]
                    if (n * 4 + tile_idx) % 2 == 0:
                        nc.scalar.activation(
                            dst,
                            acc[:],
                            mybir.ActivationFunctionType.Identity,
                            bias=bias_ap,
                        )
                    else:
                        nc.vector.tensor_scalar_add(dst, acc[:], bias_ap)
                # out stores go out on the ACT HWDGE ring so they never
                # head-of-line-block the x/w loads on the SP ring.
                nc.scalar.dma_start(out_d[n].rearrange("oh o p -> o oh p"), o_sb[:])

    nc.compile()
    return nc


def _get_nc(mode=MODE):
    if mode not in _CACHE:
        _CACHE[mode] = _build(mode)
    return _CACHE[mode]


def _np_io_dtype(mode):
    return np.float16 if mode == "fp16" else np.float32


def make_in_maps(x, weight, bias, indices, t, mode=MODE):
    idx = np.asarray(indices).astype(np.int64)
    t = int(np.asarray(t))
    io = _np_io_dtype(mode)

    w_g = np.asarray(weight)[idx, t]   # [192, 256, 256] f32
    b_g = np.asarray(bias)[idx, t, 0]  # [192, 256] f32

    in_maps = []
    for c in range(N_CORES):
        s = slice(c * N_CH, (c + 1) * N_CH)
        xt_c = np.ascontiguousarray(
            np.asarray(x)[s].transpose(0, 2, 1), dtype=io
        ).reshape(N_CH, 2, P, NPT)
        wt_c = np.ascontiguousarray(w_g[s], dtype=io).reshape(N_CH, 2, P, COUT)
        bt_c = np.ascontiguousarray(b_g[s].T, dtype=np.float32).reshape(2, P, N_CH)
        in_maps.append({"xt": xt_c, "wt": wt_c, "bt": bt_c})
    return in_maps


def assemble_out(results):
    out = np.empty((N_SEL, NPT, COUT), dtype=np.float32)
    for c in range(N_CORES):
        s = slice(c * N_CH, (c + 1) * N_CH)
        out_t = results[c]["out"].astype(np.float32).reshape(N_CH, COUT, NPT)
        out[s] = out_t.transpose(0, 2, 1)
    return out


def kernel(x, weight, bias, indices, t):
    from concourse.bass_utils import run_bass_kernel_spmd

    in_maps = make_in_maps(x, weight, bias, indices, t)
    nc = _get_nc()
    res = run_bass_kernel_spmd(nc, in_maps, core_ids=list(range(N_CORES)))
    return assemble_out(res.results)


# revision 12
# speedup vs baseline: 1.0840x; 1.0840x over previous
"""AdaptiveLinearWithChannel on 8 TRN2 NeuronCores.

out[n] = x[n] @ weight[indices[n], t] + bias[indices[n], t]
  x: [192, 2048, 256] f32, weight: [256, 8, 256, 256] f32,
  bias: [256, 8, 1, 256] f32, indices: [192] int, t: scalar int
  out: [192, 2048, 256] f32

Sharding: selected-channel axis (192) split 24-per-core across 8 cores
(expert/data parallel — no collectives). The host gathers each core's 24
weight slices from the table (equivalent traffic to a device-side gather:
only the indexed slices ever move) and pre-transposes x so the contraction
axis lands on SBUF partitions.

Device kernel (per core, per channel n):
  out_t[oh*128+o, p] = sum_ih sum_i w[ih*128+i, oh*128+o] * xT[ih*128+i, p]
  - stationary operand = weight tile [i=128, o=128] (4 per channel)
  - moving operand = xT tile [i=128, p=512]
  - accumulate over ih in PSUM [128, 512] f32 (one bank), then drain to
    SBUF fused with the bias add (bias varies along PSUM partitions -> ACT
    per-partition bias / DVE tensor_scalar). Drains alternate ACT/DVE.
  - output written transposed; host untransposes.

Two precision modes (MODE):
  "fp16": x/w/out cross HBM as fp16 (half traffic, DMA-bound kernel);
          f32 PSUM accumulate, bias added in f32. ~1e-3 rel err.
  "f32r": all HBM traffic f32; PE runs the full-rate fp32 path (float32r,
          TF32-like). ~1.5e-4 rel err, ~2x slower (DMA-bound at f32 bytes).
"""

import numpy as np

MODE = "fp16"  # "fp16" | "f32r"

N_CORES = 8
N_SEL = 192
N_CH = N_SEL // N_CORES  # 24 channels per core
NPT = 2048               # points per channel
CIN = 256
COUT = 256
P = 128                  # SBUF/PSUM partitions
PC = 512                 # moving-operand chunk (one PSUM bank of f32)
N_PC = NPT // PC         # 4
X_BUFS = 4
O_BUFS = 4
W_BUFS = 4

_CACHE = {}


def _build(mode):
    import concourse.mybir as mybir
    import concourse.tile as tile
    from concourse import bacc

    f32 = mybir.dt.float32
    if mode == "fp16":
        io_dt = mybir.dt.float16   # dtype of x/w/out in DRAM and SBUF
        pe_dt = mybir.dt.float16   # dtype the PE sees
    else:
        io_dt = f32
        pe_dt = mybir.dt.float32r

    def pe_cast(ap):
        return ap.bitcast(pe_dt) if ap.dtype != pe_dt else ap

    nc = bacc.Bacc(None, target_bir_lowering=False)
    xt_d = nc.dram_tensor("xt", [N_CH, 2, P, NPT], io_dt, kind="ExternalInput")
    wt_d = nc.dram_tensor("wt", [N_CH, 2, P, COUT], io_dt, kind="ExternalInput")
    bt_d = nc.dram_tensor("bt", [2, P, N_CH], f32, kind="ExternalInput")
    out_d = nc.dram_tensor("out", [N_CH, 2, P, NPT], io_dt, kind="ExternalOutput")

    with tile.TileContext(nc) as tc:
        with (
            tc.tile_pool(name="xp", bufs=X_BUFS) as xp,
            tc.tile_pool(name="wp", bufs=W_BUFS) as wp,
            tc.tile_pool(name="bp", bufs=1) as bp,
            tc.tile_pool(name="op", bufs=O_BUFS) as op,
            tc.tile_pool(name="ps", bufs=4, space="PSUM") as ps,
        ):
            b_sb = bp.tile([P, 2, N_CH], f32, tag="b")
            nc.sync.dma_start(b_sb[:], bt_d.rearrange("oh o n -> o oh n"))

            for n in range(N_CH):
                x_sb = xp.tile([P, 2, NPT], pe_dt, tag="x")
                nc.sync.dma_start(
                    x_sb[:], pe_cast(xt_d[n].rearrange("ih i p -> i ih p"))
                )
                w_sb = wp.tile([P, 2, COUT], pe_dt, tag="w")
                nc.sync.dma_start(
                    w_sb[:], pe_cast(wt_d[n].rearrange("ih i o -> i ih o"))
                )
                o_sb = op.tile([P, 2, NPT], io_dt, tag="o")
                # 4 two-bank PSUM tiles per channel; each holds 1024 output
                # columns of one o-half. Drains alternate ACT/DVE so both
                # engines share every wave.
                for tile_idx, (oh, pch) in enumerate(
                    [(0, 0), (0, 1), (1, 0), (1, 1)]
                ):
                    bias_ap = b_sb[:, oh, n : n + 1]
                    acc = ps.tile([P, 2 * PC], f32, tag="acc")
                    for pc2 in range(2):
                        pcg = pch * 2 + pc2
                        for ih in range(2):
                            nc.tensor.matmul(
                                acc[:, pc2 * PC : (pc2 + 1) * PC],
                                w_sb[:, ih, oh * P : (oh + 1) * P],
                                x_sb[:, ih, pcg * PC : (pcg + 1) * PC],
                                start=(ih == 0),
                                stop=(ih == 1),
                            )
                    dst = o_sb[:, oh, pch * 2 * PC : (pch + 1) * 2 * PC]
                    if (n * 4 + tile_idx) % 2 == 0:
                        nc.scalar.activation(
                            dst,
                            acc[:],
                            mybir.ActivationFunctionType.Identity,
                            bias=bias_ap,
                        )
                    else:
                        nc.vector.tensor_scalar_add(dst, acc[:], bias_ap)
                # out stores go out on the ACT HWDGE ring so they never
                # head-of-line-block the x/w loads on the SP ring.
                # (gpsimd/SWDGE stores here crash the device: NRT 101.)
                nc.scalar.dma_start(out_d[n].rearrange("oh o p -> o oh p"), o_sb[:])

    nc.compile()
    return nc


def _get_nc(mode=MODE):
    if mode not in _CACHE:
        _CACHE[mode] = _build(mode)
    return _CACHE[mode]


def _np_io_dtype(mode):
    return np.float16 if mode == "fp16" else np.float32


def make_in_maps(x, weight, bias, indices, t, mode=MODE):
    idx = np.asarray(indices).astype(np.int64)
    t = int(np.asarray(t))
    io = _np_io_dtype(mode)

    w_g = np.asarray(weight)[idx, t]   # [192, 256, 256] f32
    b_g = np.asarray(bias)[idx, t, 0]  # [192, 256] f32

    in_maps = []
    for c in range(N_CORES):
        s = slice(c * N_CH, (c + 1) * N_CH)
        xt_c = np.ascontiguousarray(
            np.asarray(x)[s].transpose(0, 2, 1), dtype=io
        ).reshape(N_CH, 2, P, NPT)
        wt_c = np.ascontiguousarray(w_g[s], dtype=io).reshape(N_CH, 2, P, COUT)
        bt_c = np.ascontiguousarray(b_g[s].T, dtype=np.float32).reshape(2, P, N_CH)
        in_maps.append({"xt": xt_c, "wt": wt_c, "bt": bt_c})
    return in_maps


def assemble_out(results):
    out = np.empty((N_SEL, NPT, COUT), dtype=np.float32)
    for c in range(N_CORES):
        s = slice(c * N_CH, (c + 1) * N_CH)
        out_t = results[c]["out"].astype(np.float32).reshape(N_CH, COUT, NPT)
        out[s] = out_t.transpose(0, 2, 1)
    return out


def kernel(x, weight, bias, indices, t):
    from concourse.bass_utils import run_bass_kernel_spmd

    in_maps = make_in_maps(x, weight, bias, indices, t)
    nc = _get_nc()
    res = run_bass_kernel_spmd(nc, in_maps, core_ids=list(range(N_CORES)))
    return assemble_out(res.results)


# revision 13
# speedup vs baseline: 1.0938x; 1.0090x over previous
"""AdaptiveLinearWithChannel on 8 TRN2 NeuronCores.

out[n] = x[n] @ weight[indices[n], t] + bias[indices[n], t]
  x: [192, 2048, 256] f32, weight: [256, 8, 256, 256] f32,
  bias: [256, 8, 1, 256] f32, indices: [192] int, t: scalar int
  out: [192, 2048, 256] f32

Sharding: selected-channel axis (192) split 24-per-core across 8 cores
(expert/data parallel — no collectives). The host gathers each core's 24
weight slices from the table (equivalent traffic to a device-side gather:
only the indexed slices ever move) and pre-transposes x so the contraction
axis lands on SBUF partitions.

Device kernel (per core, per channel n):
  out_t[oh*128+o, p] = sum_ih sum_i w[ih*128+i, oh*128+o] * xT[ih*128+i, p]
  - stationary operand = weight tile [i=128, o=128] (4 per channel)
  - moving operand = xT tile [i=128, p=512]
  - accumulate over ih into a 2-bank PSUM tile [128, 1024] f32, then drain
    to SBUF fused with the bias add (bias varies along PSUM partitions ->
    ACT per-partition activation bias / DVE tensor_scalar). Drains
    alternate ACT/DVE per tile so both engines share every wave.
  - output written transposed; host untransposes.

DMA layout (the kernel is DMA-bound: ~53.5 MB/core over ~716 GB/s per
HBM-stack pair => ~149 us floor):
  - x loads + bulk weight load on the SP HWDGE ring, out stores on the
    ACT HWDGE ring (separate FIFOs -> no head-of-line blocking between
    loads and stores). gpsimd/SWDGE stores crash the device (NRT 101).
  - all 24 channels' weights come in one bulk DMA (split 4/20 so channel
    0's matmuls start ~5 us in, not after the whole 3 MB).

Two precision modes (MODE):
  "fp16": x/w/out cross HBM as fp16 (half traffic); f32 PSUM accumulate,
          bias added in f32. ~3.6e-4 rel err, ~149 us.
  "f32r": all HBM traffic f32; PE runs the full-rate fp32 path (float32r,
          TF32-like). ~1.5e-4 rel err, ~294 us (DMA-bound at f32 bytes).
"""

import numpy as np

MODE = "fp16"  # "fp16" | "f32r"

N_CORES = 8
N_SEL = 192
N_CH = N_SEL // N_CORES  # 24 channels per core
NPT = 2048               # points per channel
CIN = 256
COUT = 256
P = 128                  # SBUF/PSUM partitions
PC = 512                 # moving-operand chunk (one PSUM bank of f32)
W_SPLIT = 4              # channels of weights in the early chunk
X_BUFS = 4
O_BUFS = 4

_CACHE = {}


def _build(mode):
    import concourse.mybir as mybir
    import concourse.tile as tile
    from concourse import bacc

    f32 = mybir.dt.float32
    if mode == "fp16":
        io_dt = mybir.dt.float16   # dtype of x/w/out in DRAM and SBUF
        pe_dt = mybir.dt.float16   # dtype the PE sees
    else:
        io_dt = f32
        pe_dt = mybir.dt.float32r

    def pe_cast(ap):
        return ap.bitcast(pe_dt) if ap.dtype != pe_dt else ap

    nc = bacc.Bacc(None, target_bir_lowering=False)
    xt_d = nc.dram_tensor("xt", [N_CH, 2, P, NPT], io_dt, kind="ExternalInput")
    wt_d = nc.dram_tensor("wt", [N_CH, 2, P, COUT], io_dt, kind="ExternalInput")
    bt_d = nc.dram_tensor("bt", [2, P, N_CH], f32, kind="ExternalInput")
    out_d = nc.dram_tensor("out", [N_CH, 2, P, NPT], io_dt, kind="ExternalOutput")

    with tile.TileContext(nc) as tc:
        with (
            tc.tile_pool(name="xp", bufs=X_BUFS) as xp,
            tc.tile_pool(name="bp", bufs=1) as bp,
            tc.tile_pool(name="op", bufs=O_BUFS) as op,
            tc.tile_pool(name="ps", bufs=4, space="PSUM") as ps,
        ):
            b_sb = bp.tile([P, 2, N_CH], f32, tag="b")
            w_sb = bp.tile([P, N_CH, 2, COUT], pe_dt, tag="w")

            def load_x(n):
                x_sb = xp.tile([P, 2, NPT], pe_dt, tag="x")
                nc.sync.dma_start(
                    x_sb[:], pe_cast(xt_d[n].rearrange("ih i p -> i ih p"))
                )
                return x_sb

            # Startup order on the SP ring: x0, w[0:4], b, x1, w[4:24] —
            # channel 0 can start computing after ~1.5 MB instead of ~4.5.
            x_tiles = {0: load_x(0)}
            nc.sync.dma_start(
                w_sb[:, :W_SPLIT],
                pe_cast(wt_d[:W_SPLIT].rearrange("n ih i o -> i n ih o")),
            )
            nc.sync.dma_start(b_sb[:], bt_d.rearrange("oh o n -> o oh n"))
            x_tiles[1] = load_x(1)
            nc.sync.dma_start(
                w_sb[:, W_SPLIT:],
                pe_cast(wt_d[W_SPLIT:].rearrange("n ih i o -> i n ih o")),
            )

            for n in range(N_CH):
                x_sb = x_tiles.pop(n) if n in x_tiles else load_x(n)
                o_sb = op.tile([P, 2, NPT], io_dt, tag="o")
                for tile_idx, (oh, pch) in enumerate(
                    [(0, 0), (0, 1), (1, 0), (1, 1)]
                ):
                    bias_ap = b_sb[:, oh, n : n + 1]
                    acc = ps.tile([P, 2 * PC], f32, tag="acc")
                    for pc2 in range(2):
                        pcg = pch * 2 + pc2
                        for ih in range(2):
                            nc.tensor.matmul(
                                acc[:, pc2 * PC : (pc2 + 1) * PC],
                                w_sb[:, n, ih, oh * P : (oh + 1) * P],
                                x_sb[:, ih, pcg * PC : (pcg + 1) * PC],
                                start=(ih == 0),
                                stop=(ih == 1),
                            )
                    dst = o_sb[:, oh, pch * 2 * PC : (pch + 1) * 2 * PC]
                    if (n * 4 + tile_idx) % 2 == 0:
                        nc.scalar.activation(
                            dst,
                            acc[:],
                            mybir.ActivationFunctionType.Identity,
                            bias=bias_ap,
                        )
                    else:
                        nc.vector.tensor_scalar_add(dst, acc[:], bias_ap)
                # out stores go on the ACT HWDGE ring so they never
                # head-of-line-block the x/w loads on the SP ring.
                nc.scalar.dma_start(out_d[n].rearrange("oh o p -> o oh p"), o_sb[:])

    nc.compile()
    return nc


def _get_nc(mode=MODE):
    if mode not in _CACHE:
        _CACHE[mode] = _build(mode)
    return _CACHE[mode]


def _np_io_dtype(mode):
    return np.float16 if mode == "fp16" else np.float32


def make_in_maps(x, weight, bias, indices, t, mode=MODE):
    idx = np.asarray(indices).astype(np.int64)
    t = int(np.asarray(t))
    io = _np_io_dtype(mode)

    w_g = np.asarray(weight)[idx, t]   # [192, 256, 256] f32
    b_g = np.asarray(bias)[idx, t, 0]  # [192, 256] f32

    in_maps = []
    for c in range(N_CORES):
        s = slice(c * N_CH, (c + 1) * N_CH)
        xt_c = np.ascontiguousarray(
            np.asarray(x)[s].transpose(0, 2, 1), dtype=io
        ).reshape(N_CH, 2, P, NPT)
        wt_c = np.ascontiguousarray(w_g[s], dtype=io).reshape(N_CH, 2, P, COUT)
        bt_c = np.ascontiguousarray(b_g[s].T, dtype=np.float32).reshape(2, P, N_CH)
        in_maps.append({"xt": xt_c, "wt": wt_c, "bt": bt_c})
    return in_maps


def assemble_out(results):
    out = np.empty((N_SEL, NPT, COUT), dtype=np.float32)
    for c in range(N_CORES):
        s = slice(c * N_CH, (c + 1) * N_CH)
        out_t = results[c]["out"].astype(np.float32).reshape(N_CH, COUT, NPT)
        out[s] = out_t.transpose(0, 2, 1)
    return out


def kernel(x, weight, bias, indices, t):
    from concourse.bass_utils import run_bass_kernel_spmd

    in_maps = make_in_maps(x, weight, bias, indices, t)
    nc = _get_nc()
    res = run_bass_kernel_spmd(nc, in_maps, core_ids=list(range(N_CORES)))
    return assemble_out(res.results)


# revision 14
# speedup vs baseline: 1.2100x; 1.1062x over previous
"""AdaptiveLinearWithChannel on 8 TRN2 NeuronCores.

out[n] = x[n] @ weight[indices[n], t] + bias[indices[n], t]
  x: [192, 2048, 256] f32, weight: [256, 8, 256, 256] f32,
  bias: [256, 8, 1, 256] f32, indices: [192] int, t: scalar int
  out: [192, 2048, 256] f32

Sharding: selected-channel axis (192) split 24-per-core across 8 cores
(expert/data parallel — no collectives). The host gathers each core's 24
weight slices from the table (equivalent traffic to a device-side gather:
only the indexed slices ever move) and pre-transposes x so the contraction
axis lands on SBUF partitions.

Device kernel (per core, per channel n):
  out_t[oh*128+o, p] = sum_ih sum_i w[ih*128+i, oh*128+o] * xT[ih*128+i, p]
  - stationary operand = weight tile [i=128, o=128] (4 per channel)
  - moving operand = xT tile [i=128, p=512]
  - accumulate over ih into a 2-bank PSUM tile [128, 1024] f32, then drain
    to SBUF fused with the bias add (bias varies along PSUM partitions ->
    ACT per-partition activation bias / DVE tensor_scalar). Drains
    alternate ACT/DVE per tile so both engines share every wave.
  - output written transposed; host untransposes.

DMA layout (the kernel is DMA-bound: ~53.5 MB/core over ~716 GB/s per
HBM-stack pair => ~149 us floor):
  - x loads + bulk weight load on the SP HWDGE ring, out stores on the
    ACT HWDGE ring (separate FIFOs -> no head-of-line blocking between
    loads and stores). gpsimd/SWDGE stores crash the device (NRT 101).
  - all 24 channels' weights come in one bulk DMA (split 4/20 so channel
    0's matmuls start ~5 us in, not after the whole 3 MB).

Two precision modes (MODE):
  "fp16": x/w/out cross HBM as fp16 (half traffic); f32 PSUM accumulate,
          bias added in f32. ~3.6e-4 rel err, ~149 us.
  "f32r": all HBM traffic f32; PE runs the full-rate fp32 path (float32r,
          TF32-like). ~1.5e-4 rel err, ~294 us (DMA-bound at f32 bytes).
"""

import numpy as np

MODE = "fp16"  # "fp16" | "f32r"

N_CORES = 8
N_SEL = 192
N_CH = N_SEL // N_CORES  # 24 channels per core
NPT = 2048               # points per channel
CIN = 256
COUT = 256
P = 128                  # SBUF/PSUM partitions
PC = 512                 # moving-operand chunk (one PSUM bank of f32)
W_SPLIT = 4              # channels of weights in the early chunk
X_BUFS = 4
O_BUFS = 4

_CACHE = {}


def _build(mode):
    import concourse.mybir as mybir
    import concourse.tile as tile
    from concourse import bacc

    f32 = mybir.dt.float32
    if mode == "fp16":
        io_dt = mybir.dt.float16   # dtype of x/w/out in DRAM and SBUF
        pe_dt = mybir.dt.float16   # dtype the PE sees
    else:
        io_dt = f32
        pe_dt = mybir.dt.float32r

    def pe_cast(ap):
        return ap.bitcast(pe_dt) if ap.dtype != pe_dt else ap

    nc = bacc.Bacc(None, target_bir_lowering=False)
    # Layouts chosen so every x/out DMA run is 8 KB contiguous per SBUF
    # partition and the bulk w load ~24 KB: xt/out [n, i, ih, p] and
    # wt [i, n, ih, o] (halves the x/out descriptor count, cuts w's 512 B
    # descriptors 6144 -> 256; ~5 us measured).
    xt_d = nc.dram_tensor("xt", [N_CH, P, 2, NPT], io_dt, kind="ExternalInput")
    wt_d = nc.dram_tensor("wt", [P, N_CH, 2, COUT], io_dt, kind="ExternalInput")
    bt_d = nc.dram_tensor("bt", [2, P, N_CH], f32, kind="ExternalInput")
    out_d = nc.dram_tensor("out", [N_CH, P, 2, NPT], io_dt, kind="ExternalOutput")

    with tile.TileContext(nc) as tc:
        with (
            tc.tile_pool(name="xp", bufs=X_BUFS) as xp,
            tc.tile_pool(name="bp", bufs=1) as bp,
            tc.tile_pool(name="op", bufs=O_BUFS) as op,
            tc.tile_pool(name="ps", bufs=4, space="PSUM") as ps,
        ):
            b_sb = bp.tile([P, 2, N_CH], f32, tag="b")
            w_sb = bp.tile([P, N_CH, 2, COUT], pe_dt, tag="w")

            def load_x(n):
                x_sb = xp.tile([P, 2, NPT], pe_dt, tag="x")
                nc.sync.dma_start(x_sb[:], pe_cast(xt_d[n]))
                return x_sb

            # Startup order on the SP ring: x0, w[0:4], b, x1, w[4:24] —
            # channel 0 can start computing after ~1.5 MB instead of ~4.5.
            x_tiles = {0: load_x(0)}
            nc.sync.dma_start(w_sb[:, :W_SPLIT], pe_cast(wt_d[:, :W_SPLIT]))
            nc.sync.dma_start(b_sb[:], bt_d.rearrange("oh o n -> o oh n"))
            x_tiles[1] = load_x(1)
            nc.sync.dma_start(w_sb[:, W_SPLIT:], pe_cast(wt_d[:, W_SPLIT:]))

            for n in range(N_CH):
                x_sb = x_tiles.pop(n) if n in x_tiles else load_x(n)
                o_sb = op.tile([P, 2, NPT], io_dt, tag="o")
                for tile_idx, (oh, pch) in enumerate(
                    [(0, 0), (0, 1), (1, 0), (1, 1)]
                ):
                    bias_ap = b_sb[:, oh, n : n + 1]
                    acc = ps.tile([P, 2 * PC], f32, tag="acc")
                    for pc2 in range(2):
                        pcg = pch * 2 + pc2
                        for ih in range(2):
                            nc.tensor.matmul(
                                acc[:, pc2 * PC : (pc2 + 1) * PC],
                                w_sb[:, n, ih, oh * P : (oh + 1) * P],
                                x_sb[:, ih, pcg * PC : (pcg + 1) * PC],
                                start=(ih == 0),
                                stop=(ih == 1),
                            )
                    dst = o_sb[:, oh, pch * 2 * PC : (pch + 1) * 2 * PC]
                    if (n * 4 + tile_idx) % 2 == 0:
                        nc.scalar.activation(
                            dst,
                            acc[:],
                            mybir.ActivationFunctionType.Identity,
                            bias=bias_ap,
                        )
                    else:
                        nc.vector.tensor_scalar_add(dst, acc[:], bias_ap)
                # out stores go on the ACT HWDGE ring so they never
                # head-of-line-block the x/w loads on the SP ring.
                nc.scalar.dma_start(out_d[n], o_sb[:])

    nc.compile()
    return nc


def _get_nc(mode=MODE):
    if mode not in _CACHE:
        _CACHE[mode] = _build(mode)
    return _CACHE[mode]


def _np_io_dtype(mode):
    return np.float16 if mode == "fp16" else np.float32


def make_in_maps(x, weight, bias, indices, t, mode=MODE):
    idx = np.asarray(indices).astype(np.int64)
    t = int(np.asarray(t))
    io = _np_io_dtype(mode)

    w_g = np.asarray(weight)[idx, t]   # [192, 256, 256] f32
    b_g = np.asarray(bias)[idx, t, 0]  # [192, 256] f32

    in_maps = []
    for c in range(N_CORES):
        s = slice(c * N_CH, (c + 1) * N_CH)
        xt_c = np.ascontiguousarray(
            np.asarray(x)[s]
            .transpose(0, 2, 1)
            .reshape(N_CH, 2, P, NPT)
            .transpose(0, 2, 1, 3),
            dtype=io,
        )
        wt_c = np.ascontiguousarray(
            w_g[s].reshape(N_CH, 2, P, COUT).transpose(2, 0, 1, 3), dtype=io
        )
        bt_c = np.ascontiguousarray(b_g[s].T, dtype=np.float32).reshape(2, P, N_CH)
        in_maps.append({"xt": xt_c, "wt": wt_c, "bt": bt_c})
    return in_maps


def assemble_out(results):
    out = np.empty((N_SEL, NPT, COUT), dtype=np.float32)
    for c in range(N_CORES):
        s = slice(c * N_CH, (c + 1) * N_CH)
        out_t = (
            results[c]["out"]
            .astype(np.float32)
            .reshape(N_CH, P, 2, NPT)
            .transpose(0, 2, 1, 3)
            .reshape(N_CH, COUT, NPT)
        )
        out[s] = out_t.transpose(0, 2, 1)
    return out


def kernel(x, weight, bias, indices, t):
    from concourse.bass_utils import run_bass_kernel_spmd

    in_maps = make_in_maps(x, weight, bias, indices, t)
    nc = _get_nc()
    res = run_bass_kernel_spmd(nc, in_maps, core_ids=list(range(N_CORES)))
    return assemble_out(res.results)
